# revision 16
# baseline (speedup 1.0000x reference)
"""Trainium2 Bass kernel for a FlowNet-style CorrelationLayer.

out[0, j*7+i, h, w] = sum_c x[0,c,h,w] * y[0,c,h+j-3,w+i-3]   (zero-padded y)

Shapes: x, y = [1, 128, 384, 512] fp32  ->  out = [1, 49, 384, 512] fp32.

Strategy
--------
* Shard H (rows) across the 8 NeuronCores: core k computes output rows
  [48k, 48k+48).  The y halo (3 rows each side) is sliced on the host from
  the full input, so no inter-core communication is needed.
* Per core, the C=128 contraction runs on the TensorEngine as "all-pairs"
  patch matmuls: lhsT = an 8x8 pixel patch of x (M=64 columns, K=C=128),
  rhs = the matching 14x14 halo patch of y (N=196 columns).  Entry
  (m=(a,b), n=(al,be)) of the PSUM block is the correlation of x pixel
  (a,b) with y pixel (al-3, be-3) relative to the patch origin, so the 49
  shift planes live on 49 diagonals of each block.  Two patches are packed
  into the 128 PE columns via tile_position col-tiling so partitions (and
  hence DMA width) stay full.
* Diagonal extraction is not expressible with uniform per-partition access
  patterns on any engine, so each PSUM block is cast to fp16 and dumped
  whole to DRAM; the final banded gather is a cheap numpy fancy-index on
  the host.  Inputs are also shipped as fp16 (quantization error ~4e-4
  relative, well within tolerance).  Total HBM traffic per core is
  ~23 MB, close to the memory roofline.
"""

import numpy as np

import concourse.bass as bass  # noqa: F401  (AP types pulled in transitively)
import concourse.tile as tile
from concourse import bacc, mybir
from concourse.bass_utils import run_bass_kernel_spmd

B, C, H, W = 1, 128, 384, 512
NCORES = 8
HB = H // NCORES          # 48 output rows per core
PA, PB = 8, 8             # x patch: 8 rows x 8 cols = 64 = M per matmul
HA, HW_ = PA + 6, PB + 6  # y halo patch: 14 x 14
NF = HA * HW_             # 196 = N (matmul free size)
PR = HB // PA             # 6 patch-rows
PW = W // PB              # 64 patch-cols
PQ = PW // 2              # 32 pairs (two patches packed per 128 partitions)

F16 = mybir.dt.float16

NROW = 8 * HW_          # 112 halo values per pixel-row-pair (8 of 14 halo rows)
WIN = NROW * PQ         # 3584: contiguous window per partition, n-major layout
RSZ = NF * PQ           # 6272: staging row size (elements per partition)

_PROGRAM = None


def _build_program():
    nc = bacc.Bacc("TRN2", target_bir_lowering=False, debug=False)

    # x is pre-tiled on the host to [C, patch, m] so each patch's 64 weight
    # columns are contiguous (walrus requires a single free dim on the
    # stationary matmul operand).
    xb = nc.declare_dram_parameter("xb", [C, PR * PW, PA * PB], F16, isOutput=False)
    yb = nc.declare_dram_parameter("yb", [C, HB + 6, W + 6], F16, isOutput=False)
    # Compacted output: pixel-row pairs a=(2q, 2q+1) share a 112-halo-row
    # window (of 196), shipped as one contiguous slab per (q, half).
    corr = nc.declare_dram_parameter("corr", [PR, 4, 2, 16, WIN], F16, isOutput=True)

    with tile.TileContext(nc) as tc:
        with (
            tc.tile_pool(name="xpool", bufs=1) as xpool,
            tc.tile_pool(name="ypool", bufs=1) as ypool,
            tc.tile_pool(name="psum", bufs=4, space="PSUM") as psum_pool,
            tc.tile_pool(name="stage", bufs=6) as stage_pool,
        ):
            X = xpool.tile([C, PR * PW, PA * PB], F16)
            Y = ypool.tile([C, HB + 6, W + 6], F16)

            # Issue input loads in the order the patch-row pipeline consumes
            # them (the HW queue drains FIFO): patch-row pr needs X chunk pr
            # and Y rows [8pr, 8pr+14) = Y chunks pr and pr+1.
            def load_x(pr):
                nc.sync.dma_start(
                    X[:, pr * PW : (pr + 1) * PW, :], xb[:, pr * PW : (pr + 1) * PW, :]
                )

            def load_y(ch):  # Y chunk = 8 rows (last chunk 6 rows)
                r0, r1 = ch * 8, min(ch * 8 + 8, HB + 6)
                nc.sync.dma_start(Y[:, r0:r1, :], yb[:, r0:r1, :])

            load_x(0)
            load_y(0)
            load_y(1)
            for pr in range(1, PR):
                load_x(pr)
                load_y(pr + 1)

            for pr in range(PR):
                # n-major staging: element offset = n*PQ + q, so each pixel-row
                # group a's needed window (halo rows a..a+6) is one contiguous
                # 3136-element slab at [448*a, 448*a + WIN).
                st = stage_pool.tile([128, RSZ], F16)
                stv = st[:, :]
                for qq in range(0, PQ, 2):
                    # Four 8x8 patches (two col-tiled pairs) share one PSUM
                    # bank; their evacuation is a single strided copy.
                    ps = psum_pool.tile([128, 2, 256], mybir.dt.float32)
                    for s in range(2):
                        q = qq + s
                        for half in range(2):
                            wp = 2 * q + half
                            lhsT = X[:, pr * PW + wp, :]
                            rhs = Y[
                                :, pr * PA : pr * PA + HA, wp * PB : wp * PB + HW_
                            ]
                            nc.tensor.matmul(
                                ps[half * 64 : (half + 1) * 64, s, :NF],
                                lhsT,
                                rhs,
                                start=True,
                                stop=True,
                                tile_position=(0, 64 * half),
                            )
                    # Scatter-evac into the n-major layout: dst elements
                    # n*PQ + (qq+s).  Alternate DVE/ACT.
                    dst = bass.AP(
                        tensor=stv.tensor,
                        offset=stv.offset + qq,
                        ap=[[RSZ, 128], [1, 2], [PQ, NF]],
                    )
                    if (qq // 2) % 2 == 0:
                        nc.vector.tensor_copy(dst, ps[:, :, :NF])
                    else:
                        nc.scalar.copy(dst, ps[:, :, :NF])
                # Compacted stores: pixel-row pair q = (a=2q, 2q+1) occupies 16
                # contiguous partitions per half and needs halo rows
                # [28q, 28q+112) = elements [896q, 896q+WIN) -- a plain 2-dim
                # slice.  Rows 0..4 queue on the Sync ring BEHIND the loads
                # (FIFO), so loads never lose bandwidth to stores; the last
                # row's stores go on the idle ACT ring to dodge the queue.
                for q in range(4):
                    for h in range(2):
                        src = st[
                            64 * h + 16 * q : 64 * h + 16 * q + 16,
                            896 * q : 896 * q + WIN,
                        ]
                        eng = nc.scalar if pr == PR - 1 else nc.sync
                        eng.dma_start(corr[pr, q, h], src)

    nc.compile()
    return nc


def _program():
    global _PROGRAM
    if _PROGRAM is None:
        _PROGRAM = _build_program()
    return _PROGRAM


def _make_in_maps(x: np.ndarray, y: np.ndarray):
    x0 = np.asarray(x[0]).astype(np.float16)
    # [C, H, W] -> [C, H/PA, PA, PW, PB] -> [C, H/PA, PW, PA, PB]
    xt = x0.reshape(C, H // PA, PA, PW, PB).transpose(0, 1, 3, 2, 4)
    xt = np.ascontiguousarray(xt.reshape(C, H // PA * PW, PA * PB))
    yp = np.zeros((C, H + 6, W + 6), np.float16)
    yp[:, 3 : 3 + H, 3 : 3 + W] = y[0]
    in_maps = []
    for k in range(NCORES):
        in_maps.append(
            {
                "xb": np.ascontiguousarray(xt[:, k * PR * PW : (k + 1) * PR * PW, :]),
                "yb": np.ascontiguousarray(yp[:, k * HB : k * HB + HB + 6, :]),
            }
        )
    return in_maps


_GATHER_IDX = None


def _gather_indices():
    global _GATHER_IDX
    if _GATHER_IDX is None:
        v = np.arange(2)[:, None, None, None]
        b = np.arange(PB)[None, :, None, None]
        j = np.arange(7)[None, None, :, None]
        i = np.arange(7)[None, None, None, :]
        # within-window halo-row index for pixel (a = 2q+v, b), shift (j, i):
        # n - 28q = 14v + 14j + (b + i)
        n_idx = (14 * v + 14 * j + b + i).reshape(1, 1, 1, 2, PB, 49, 1)
        _GATHER_IDX = np.ascontiguousarray(n_idx)
    return _GATHER_IDX


def _gather_core(corr_k: np.ndarray) -> np.ndarray:
    """[PR, 4, 2, 16, WIN] -> [49, HB, W] band of the output."""
    n_idx = _gather_indices()
    ck = corr_k.reshape(PR, 4, 2, 2, PB, 8 * HW_, PQ)
    g = np.take_along_axis(ck, n_idx, axis=5)  # [pr, q, h, v, b, 49, c]
    # out[s, pr*8 + 2q + v, (2c+h)*8 + b] = g[pr, q, h, v, b, s, c]
    g = g.transpose(5, 0, 1, 3, 6, 2, 4).reshape(49, HB, W)
    return g


def _run(in_maps, trace=False, **kw):
    return run_bass_kernel_spmd(
        _program(), in_maps, core_ids=list(range(NCORES)), trace=trace, **kw
    )


def kernel(x: np.ndarray, y: np.ndarray) -> np.ndarray:
    x = np.asarray(x)
    y = np.asarray(y)
    res = _run(_make_in_maps(x, y)).results
    out = np.empty((1, 49, H, W), np.float32)
    for k in range(NCORES):
        out[0, :, k * HB : (k + 1) * HB, :] = _gather_core(
            np.asarray(res[k]["corr"])
        ).astype(np.float32)
    return out



# revision 20
# speedup vs baseline: 1.5304x; 1.5304x over previous
"""Trainium2 Bass kernel for a FlowNet-style CorrelationLayer.

out[0, j*7+i, h, w] = sum_c x[0,c,h,w] * y[0,c,h+j-3,w+i-3]   (zero-padded y)

Shapes: x, y = [1, 128, 384, 512] fp32  ->  out = [1, 49, 384, 512] fp32.

Strategy
--------
* Shard H (rows) across the 8 NeuronCores: core k computes output rows
  [48k, 48k+48).  The y halo (3 rows each side) is sliced on the host from
  the full input, so no inter-core communication is needed.
* Per core, the C=128 contraction runs on the TensorEngine as "all-pairs"
  patch matmuls: lhsT = an 8x8 pixel patch of x (M=64 columns, K=C=128),
  rhs = the matching 14x14 halo patch of y (N=196 columns).  Entry
  (m=(a,b), n=(al,be)) of the PSUM block is the correlation of x pixel
  (a,b) with y pixel (al-3, be-3) relative to the patch origin, so the 49
  shift planes live on 49 diagonals of each block.  Two patches are packed
  into the 128 PE columns via tile_position col-tiling so partitions (and
  hence DMA width) stay full.
* Diagonal extraction is not expressible with uniform per-partition access
  patterns on any engine, so each PSUM block is cast to fp16 and dumped
  whole to DRAM; the final banded gather is a cheap numpy fancy-index on
  the host.  Inputs are also shipped as fp16 (quantization error ~4e-4
  relative, well within tolerance).  Total HBM traffic per core is
  ~23 MB, close to the memory roofline.
"""

import numpy as np

import concourse.bass as bass  # noqa: F401  (AP types pulled in transitively)
import concourse.tile as tile
from concourse import bacc, mybir
from concourse.bass_utils import run_bass_kernel_spmd

B, C, H, W = 1, 128, 384, 512
NCORES = 8
HB = H // NCORES          # 48 output rows per core
PA, PB = 8, 8             # x patch: 8 rows x 8 cols = 64 = M per matmul
HA, HW_ = PA + 6, PB + 6  # y halo patch: 14 x 14
NF = HA * HW_             # 196 = N (matmul free size)
PR = HB // PA             # 6 patch-rows
PW = W // PB              # 64 patch-cols
PQ = PW // 2              # 32 pairs (two patches packed per 128 partitions)

F16 = mybir.dt.float16

NROW = 8 * HW_          # 112 halo values per pixel-row-pair (8 of 14 halo rows)
WIN = NROW * PQ         # 3584: contiguous window per partition, n-major layout
RSZ = NF * PQ           # 6272: staging row size (elements per partition)

_PROGRAM = None


def _build_program():
    nc = bacc.Bacc("TRN2", target_bir_lowering=False, debug=False)

    # x is pre-tiled on the host to [C, patch, m] so each patch's 64 weight
    # columns are contiguous (walrus requires a single free dim on the
    # stationary matmul operand).
    xb = nc.declare_dram_parameter("xb", [C, PR * PW, PA * PB], F16, isOutput=False)
    yb = nc.declare_dram_parameter("yb", [C, HB + 6, W + 6], F16, isOutput=False)
    # Compacted output: pixel-row pairs a=(2q, 2q+1) share a 112-halo-row
    # window (of 196), shipped as one contiguous slab per (q, half).
    corr = nc.declare_dram_parameter("corr", [PR, 4, 2, 16, WIN], F16, isOutput=True)

    with tile.TileContext(nc) as tc:
        with (
            tc.tile_pool(name="xpool", bufs=1) as xpool,
            tc.tile_pool(name="ypool", bufs=1) as ypool,
            tc.tile_pool(name="psum", bufs=4, space="PSUM") as psum_pool,
            tc.tile_pool(name="stage", bufs=6) as stage_pool,
        ):
            X = xpool.tile([C, PR * PW, PA * PB], F16)
            Y = ypool.tile([C, HB + 6, W + 6], F16)

            # Issue input loads in the order the patch-row pipeline consumes
            # them (the HW queue drains FIFO): patch-row pr needs X chunk pr
            # and Y rows [8pr, 8pr+14) = Y chunks pr and pr+1.
            def load_x(pr):
                nc.sync.dma_start(
                    X[:, pr * PW : (pr + 1) * PW, :], xb[:, pr * PW : (pr + 1) * PW, :]
                )

            def load_y(ch):  # Y chunk = 8 rows (last chunk 6 rows)
                r0, r1 = ch * 8, min(ch * 8 + 8, HB + 6)
                nc.sync.dma_start(Y[:, r0:r1, :], yb[:, r0:r1, :])

            load_x(0)
            load_y(0)
            load_y(1)
            for pr in range(1, PR):
                load_x(pr)
                load_y(pr + 1)

            for pr in range(PR):
                # n-major staging: element offset = n*PQ + q, so each pixel-row
                # group a's needed window (halo rows a..a+6) is one contiguous
                # 3136-element slab at [448*a, 448*a + WIN).
                st = stage_pool.tile([128, RSZ], F16)
                stv = st[:, :]
                for qq in range(0, PQ, 2):
                    # Four 8x8 patches (two col-tiled pairs) share one PSUM
                    # bank; their evacuation is a single strided copy.
                    ps = psum_pool.tile([128, 2, 256], mybir.dt.float32)
                    for s in range(2):
                        q = qq + s
                        for half in range(2):
                            wp = 2 * q + half
                            lhsT = X[:, pr * PW + wp, :]
                            rhs = Y[
                                :, pr * PA : pr * PA + HA, wp * PB : wp * PB + HW_
                            ]
                            nc.tensor.matmul(
                                ps[half * 64 : (half + 1) * 64, s, :NF],
                                lhsT,
                                rhs,
                                start=True,
                                stop=True,
                                tile_position=(0, 64 * half),
                            )
                    # Scatter-evac into the block-n-major layout: element
                    # offset = nb*896 + q*28 + nk with n = 28*nb + nk, so the
                    # write runs are 28 contiguous elements (fast) and each
                    # a-pair's store window stays one contiguous slab.  One
                    # copy per s keeps both APs 3-dim (4-dim PSUM-source
                    # copies scramble data); the two copies of a tile go to
                    # different engines.
                    for s in range(2):
                        dst = bass.AP(
                            tensor=stv.tensor,
                            offset=stv.offset + (qq + s) * 28,
                            ap=[[RSZ, 128], [896, 7], [1, 28]],
                        )
                        src = ps[:, s, :NF].rearrange(
                            "p (nb nk) -> p nb nk", nb=7
                        )
                        if (qq // 2 + s) % 2 == 0:
                            nc.vector.tensor_copy(dst, src)
                        else:
                            nc.scalar.copy(dst, src)
                # Compacted stores: pixel-row pair q = (a=2q, 2q+1) occupies 16
                # contiguous partitions per half and needs halo rows
                # [28q, 28q+112) = elements [896q, 896q+WIN) -- a plain 2-dim
                # slice.  Rows 0..4 queue on the Sync ring BEHIND the loads
                # (FIFO), so loads never lose bandwidth to stores; the last
                # row's stores go on the idle ACT ring to dodge the queue.
                for q in range(4):
                    for h in range(2):
                        src = st[
                            64 * h + 16 * q : 64 * h + 16 * q + 16,
                            896 * q : 896 * q + WIN,
                        ]
                        eng = nc.scalar if pr == PR - 1 else nc.sync
                        eng.dma_start(corr[pr, q, h], src)

    nc.compile()
    return nc


def _program():
    global _PROGRAM
    if _PROGRAM is None:
        _PROGRAM = _build_program()
    return _PROGRAM


def _make_in_maps(x: np.ndarray, y: np.ndarray):
    x0 = np.asarray(x[0]).astype(np.float16)
    # [C, H, W] -> [C, H/PA, PA, PW, PB] -> [C, H/PA, PW, PA, PB]
    xt = x0.reshape(C, H // PA, PA, PW, PB).transpose(0, 1, 3, 2, 4)
    xt = np.ascontiguousarray(xt.reshape(C, H // PA * PW, PA * PB))
    yp = np.zeros((C, H + 6, W + 6), np.float16)
    yp[:, 3 : 3 + H, 3 : 3 + W] = y[0]
    in_maps = []
    for k in range(NCORES):
        in_maps.append(
            {
                "xb": np.ascontiguousarray(xt[:, k * PR * PW : (k + 1) * PR * PW, :]),
                "yb": np.ascontiguousarray(yp[:, k * HB : k * HB + HB + 6, :]),
            }
        )
    return in_maps


_GATHER_IDX = None


def _gather_indices():
    global _GATHER_IDX
    if _GATHER_IDX is None:
        v = np.arange(2)[:, None, None, None, None]
        b = np.arange(PB)[None, :, None, None, None]
        j = np.arange(7)[None, None, :, None, None]
        i = np.arange(7)[None, None, None, :, None]
        c = np.arange(PQ)[None, None, None, None, :]
        # within-window halo-row index for pixel (a = 2Q+v, b), shift (j, i):
        # n' = 14v + 14j + (b + i); slab element (block-n-major) =
        # (n'//28)*896 + c*28 + (n'%28)
        np_ = 14 * v + 14 * j + b + i
        e = (np_ // 28) * 896 + c * 28 + (np_ % 28)
        _GATHER_IDX = np.ascontiguousarray(
            e.reshape(2, PB, 49 * PQ).reshape(1, 1, 1, 2, PB, 49 * PQ)
        )
    return _GATHER_IDX


def _gather_core(corr_k: np.ndarray) -> np.ndarray:
    """[PR, 4, 2, 16, WIN] -> [49, HB, W] band of the output."""
    n_idx = _gather_indices()
    ck = corr_k.reshape(PR, 4, 2, 2, PB, WIN)
    g = np.take_along_axis(ck, n_idx, axis=5)  # [pr, Q, h, v, b, 49*PQ]
    g = g.reshape(PR, 4, 2, 2, PB, 49, PQ)
    # out[s, pr*8 + 2Q + v, (2c+h)*8 + b] = g[pr, Q, h, v, b, s, c]
    g = g.transpose(5, 0, 1, 3, 6, 2, 4).reshape(49, HB, W)
    return g


def _run(in_maps, trace=False, **kw):
    return run_bass_kernel_spmd(
        _program(), in_maps, core_ids=list(range(NCORES)), trace=trace, **kw
    )


def kernel(x: np.ndarray, y: np.ndarray) -> np.ndarray:
    x = np.asarray(x)
    y = np.asarray(y)
    res = _run(_make_in_maps(x, y)).results
    out = np.empty((1, 49, H, W), np.float32)
    for k in range(NCORES):
        out[0, :, k * HB : (k + 1) * HB, :] = _gather_core(
            np.asarray(res[k]["corr"])
        ).astype(np.float32)
    return out

